# revision 6
# baseline (speedup 1.0000x reference)
"""Causal multi-head attention block (B=4, S=2048, D=768, H=12, Dh=64)
distributed over 8 NeuronCores: core = (batch, head-group), each core
computes its 6 heads end-to-end plus its partial output projection;
host sums the two partials per batch and adds the bias.

Self-contained: hardcodes all shapes; no sibling imports.
"""

import numpy as np

B, S, D = 4, 2048, 768
H, DH = 12, 64
G = 384          # channels per head group (6 heads)
NPAIR = 3        # head pairs per core
NSC = 4          # 512-wide query windows
W = 512
NST = 16         # 128-row s-tiles
NDC = 6          # 128-row D chunks

_PROGRAM = None
PROFILE = False
PROFILE_DIR = None
LAST_RESULT = None


def _split_waits(nc, max_waits=1, max_updates=1):
    """This container's walrus rejects instructions carrying more than one
    semaphore wait/update ("Too many sync wait commands").  Move excess
    waits onto NoOps inserted before the owning instruction (same engine)
    and excess updates onto NoOps inserted after."""
    import concourse.mybir as mybir

    counter = [0]

    def nop(engine, waits, updates):
        counter[0] += 1
        n = mybir.InstNoOp(name=f"wsplit_nop_{counter[0]}", ins=[], outs=[])
        n.engine = engine
        n.sync_info = mybir.SyncInfo(on_wait=waits, on_update=updates)
        return n

    for bb in nc.main_func.blocks:
        out = []
        changed = False
        for ins in bb.instructions:
            si = ins.sync_info
            waits = list(si.on_wait) if si and si.on_wait else []
            updates = list(si.on_update) if si and si.on_update else []
            pre, post = [], []
            if len(waits) > max_waits:
                keep = waits[:max_waits - 1] if max_waits > 1 else []
                rest = waits[len(keep):]
                while rest:
                    chunk, rest = rest[:max_waits], rest[max_waits:]
                    pre.append(chunk)
                waits = keep
                changed = True
            if len(updates) > max_updates:
                rest = updates[max_updates:]
                updates = updates[:max_updates]
                while rest:
                    chunk, rest = rest[:max_updates], rest[max_updates:]
                    post.append(chunk)
                changed = True
            if pre or post:
                ins.sync_info = mybir.SyncInfo(
                    on_wait=waits, on_update=updates)
            for w in pre:
                out.append(nop(ins.engine, w, []))
            out.append(ins)
            for u in post:
                out.append(nop(ins.engine, [], u))
        if changed:
            bb.instructions = out


def _install_profile_hooks():
    """Dev-only (PROFILE=True): register the NTFF profile hook that the
    agent image's antenv lacks, and stub out the artifact upload."""
    import sys
    import types

    try:
        from antenv.axon_hooks import get_axon_ntff_profile_hook  # noqa: F401
    except ImportError:
        import antenv
        from trn_agent_boot import trn_boot

        hook = trn_boot._ntff_profile_via_ctypes("/opt/axon/libaxon_pjrt.so")
        mod = types.ModuleType("antenv.axon_hooks")
        mod._hook = hook
        mod.get_axon_ntff_profile_hook = lambda: mod._hook
        mod.set_axon_ntff_profile_hook = lambda h: setattr(mod, "_hook", h)
        sys.modules["antenv.axon_hooks"] = mod
        antenv.axon_hooks = mod

    from concourse import bass_utils

    bass_utils.upload_artifacts = lambda tmpdir: "local://" + tmpdir


def _build_program():
    import concourse.bass as bass
    import concourse.mybir as mybir
    import concourse.tile as tile

    f16 = mybir.dt.float16
    f32 = mybir.dt.float32

    nc = bass.Bass()
    xt_d = nc.declare_dram_parameter("xt", [D, S], f16, isOutput=False)
    wq_d = nc.declare_dram_parameter("wq", [D, G], f16, isOutput=False)
    wk_d = nc.declare_dram_parameter("wk", [D, G], f16, isOutput=False)
    wv_d = nc.declare_dram_parameter("wv", [D, G], f16, isOutput=False)
    wo_d = nc.declare_dram_parameter("wo", [G, D], f16, isOutput=False)
    mk_d = nc.declare_dram_parameter("mk", [128, 128], f16, isOutput=False)
    y_d = nc.declare_dram_parameter("y", [S, D], f16, isOutput=True)

    with tile.TileContext(nc) as tc:
        with (
            tc.tile_pool(name="const", bufs=1) as const,
            tc.tile_pool(name="work", bufs=3) as work,
            tc.tile_pool(name="outp", bufs=3) as outp,
            tc.tile_pool(name="ps", bufs=2, space="PSUM") as ps,
        ):
            # ---- persistent SBUF tiles ----
            xt = [const.tile([128, S], f16, name=f"xt{i}", tag=f"xt{i}")
                  for i in range(NDC)]
            wq = [const.tile([128, G], f16, name=f"wq{i}", tag=f"wq{i}")
                  for i in range(NDC)]
            wk = [const.tile([128, G], f16, name=f"wk{i}", tag=f"wk{i}")
                  for i in range(NDC)]
            wv = [const.tile([128, G], f16, name=f"wv{i}", tag=f"wv{i}")
                  for i in range(NDC)]
            wo = [const.tile([128, D], f16, name=f"wo{i}", tag=f"wo{i}")
                  for i in range(3)]
            qt = [const.tile([128, S], f16, name=f"qt{p}", tag=f"qt{p}")
                  for p in range(NPAIR)]
            kt = [const.tile([128, S], f16, name=f"kt{p}", tag=f"kt{p}")
                  for p in range(NPAIR)]
            vt = [const.tile([128, G], f16, name=f"vt{t}", tag=f"vt{t}")
                  for t in range(NST)]
            gt = [const.tile([128, S], f16, name=f"gt{p}", tag=f"gt{p}")
                  for p in range(NPAIR)]
            mk = const.tile([128, 128], f16, name="mk", tag="mk")
            ones = const.tile([128, DH], f16, name="ones", tag="ones")

            # ---- input DMAs, spread over three queues (sync + scalar HW
            # DGE, gpsimd SW DGE) in need-by order: window-0 inputs (wq,
            # xt cols 0:512, wk, mk) first so the first projection starts
            # ~6us in; later windows' columns stream behind them. ----
            def xq(i, a, b):  # xt column slice loader
                return dict(out=xt[i][:, a:b],
                            in_=xt_d[128 * i:128 * (i + 1), a:b])

            nc.sync.dma_start(out=mk, in_=mk_d[:, :])
            for i in range(NDC):
                nc.sync.dma_start(**xq(i, 0, 512))
                nc.scalar.dma_start(out=wq[i], in_=wq_d[128 * i:128 * (i + 1), :])
                nc.gpsimd.dma_start(out=wk[i], in_=wk_d[128 * i:128 * (i + 1), :])
            for i in range(NDC):
                nc.scalar.dma_start(out=wv[i], in_=wv_d[128 * i:128 * (i + 1), :])
                nc.gpsimd.dma_start(**xq(i, 512, 1024))
            for i in range(NDC):
                nc.sync.dma_start(**xq(i, 1024, 1536))
                nc.scalar.dma_start(**xq(i, 1536, 2048))
            for i in range(3):
                nc.gpsimd.dma_start(out=wo[i], in_=wo_d[128 * i:128 * (i + 1), :])
            nc.vector.memset(ones, 1.0)

            def act_recip(out, in_):
                # ScalarE table reciprocal (~1e-5 rel err on [1e-2, 1e7],
                # verified on HW) -- keeps the softmax divide off the DVE
                # and off the inter-window critical path.
                eng = nc.scalar
                ins_ = [eng.lower_ap(in_[:, :]),
                        mybir.ImmediateValue(dtype=mybir.dt.float32, value=0.0),
                        mybir.ImmediateValue(dtype=mybir.dt.float32, value=1.0),
                        mybir.ImmediateValue(dtype=mybir.dt.float32, value=0.0)]
                eng.add_instruction(mybir.InstActivation(
                    name=nc.get_next_instruction_name(),
                    func=mybir.ActivationFunctionType.Reciprocal,
                    ins=ins_, outs=[eng.lower_ap(out[:, :])]))

            def proj_qk_unit(pair, sc):
                qp = ps.tile([128, W], f32, name=f"qp{pair}_{sc}",
                             tag="sc", bufs=2)
                for dc in range(NDC):
                    nc.tensor.matmul(
                        qp,
                        wq[dc][:, 128 * pair:128 * (pair + 1)],
                        xt[dc][:, W * sc:W * (sc + 1)],
                        start=(dc == 0), stop=(dc == NDC - 1))
                nc.vector.tensor_copy(
                    out=qt[pair][:, W * sc:W * (sc + 1)], in_=qp)
                kp = ps.tile([128, W], f32, name=f"kp{pair}_{sc}",
                             tag="sc", bufs=2)
                for dc in range(NDC):
                    nc.tensor.matmul(
                        kp,
                        wk[dc][:, 128 * pair:128 * (pair + 1)],
                        xt[dc][:, W * sc:W * (sc + 1)],
                        start=(dc == 0), stop=(dc == NDC - 1))
                nc.vector.tensor_copy(
                    out=kt[pair][:, W * sc:W * (sc + 1)], in_=kp)

            def proj_v(st):
                vp = ps.tile([128, G], f32, name=f"vp{st}", tag="sc", bufs=2)
                for dc in range(NDC):
                    nc.tensor.matmul(
                        vp,
                        xt[dc][:, 128 * st:128 * (st + 1)],
                        wv[dc],
                        start=(dc == 0), stop=(dc == NDC - 1))
                nc.vector.tensor_copy(out=vt[st], in_=vp)

            def outproj(st):
                o0 = ps.tile([128, G], f32, name=f"o0_{st}", tag="apv", bufs=2)
                for cc in range(3):
                    nc.tensor.matmul(
                        o0,
                        gt[cc][:, 128 * st:128 * (st + 1)],
                        wo[cc][:, 0:G],
                        start=(cc == 0), stop=(cc == 2))
                o1 = ps.tile([128, G], f32, name=f"o1_{st}", tag="adn", bufs=2)
                for cc in range(3):
                    nc.tensor.matmul(
                        o1,
                        gt[cc][:, 128 * st:128 * (st + 1)],
                        wo[cc][:, G:D],
                        start=(cc == 0), stop=(cc == 2))
                ob = outp.tile([128, D], f16, name=f"ob{st}", tag="ob", bufs=4)
                nc.vector.tensor_copy(out=ob[:, 0:G], in_=o0)
                nc.vector.tensor_copy(out=ob[:, G:D], in_=o1)
                eng = nc.sync if st % 2 == 0 else nc.gpsimd
                eng.dma_start(
                    out=y_d[128 * st:128 * (st + 1), :], in_=ob)

            class PairAttention:
                """Per-pair attention with a software pipeline that crosses
                window boundaries: pv/dn of group g are issued after the
                scores of group g+1 (even when g+1 is in the next query
                window), so neither the PE nor ACT drains at boundaries."""

                def __init__(self, pair):
                    self.pair = pair
                    self.prev = None

                def scores_exp(self, sc, jb):
                    pair = self.pair
                    col0 = max(0, 128 * jb - W * sc)
                    diag = jb >= 4 * sc
                    sct = ps.tile([128, 1024], f32, name=f"sc{pair}_{sc}_{jb}",
                                  tag="sc", bufs=2)
                    nc.tensor.matmul(
                        sct[:, col0:W],
                        kt[pair][0:64, 128 * jb:128 * (jb + 1)],
                        qt[pair][0:64, W * sc + col0:W * (sc + 1)],
                        start=True, stop=True)
                    nc.tensor.matmul(
                        sct[:, W:2 * W - col0],
                        kt[pair][64:128, 128 * jb:128 * (jb + 1)],
                        qt[pair][64:128, W * sc + col0:W * (sc + 1)],
                        start=True, stop=True)
                    ex = work.tile([128, 1024], f16, name=f"ex{pair}_{sc}_{jb}",
                                   tag="exp", bufs=6)
                    nc.scalar.activation(
                        out=ex[:, col0:2 * W - col0],
                        in_=sct[:, col0:2 * W - col0],
                        func=mybir.ActivationFunctionType.Exp, scale=0.125)
                    if diag:  # zero the j>i triangle of the diagonal block
                        nc.vector.tensor_mul(
                            ex[:, col0:col0 + 128], ex[:, col0:col0 + 128], mk)
                        nc.vector.tensor_mul(
                            ex[:, W:W + 128], ex[:, W:W + 128], mk)
                    return ex

                def pv_dn(self, state):
                    pair = self.pair
                    pv, dn, sc, jb, ex = state
                    col0 = max(0, 128 * jb - W * sc)
                    first, last = (jb == 0), (jb == 4 * sc + 3)
                    nc.tensor.matmul(
                        pv[0:64, col0:W],
                        vt[jb][:, 128 * pair:128 * pair + 64],
                        ex[:, col0:W],
                        start=first, stop=last)
                    nc.tensor.matmul(
                        pv[64:128, col0:W],
                        vt[jb][:, 128 * pair + 64:128 * (pair + 1)],
                        ex[:, W:2 * W - col0],
                        start=first, stop=last)
                    nc.tensor.matmul(
                        dn[0:64, col0:W],
                        ones,
                        ex[:, col0:W],
                        start=first, stop=last)
                    nc.tensor.matmul(
                        dn[64:128, col0:W],
                        ones,
                        ex[:, W:2 * W - col0],
                        start=first, stop=last)
                    if last:  # window complete: normalize into gt
                        rc = work.tile([128, W], f32, name=f"rc{pair}_{sc}",
                                       tag="rc", bufs=3)
                        act_recip(rc, dn)
                        nc.vector.tensor_mul(
                            gt[pair][:, W * sc:W * (sc + 1)], pv, rc)

                def window(self, sc, filler_tick):
                    pair = self.pair
                    pv = ps.tile([128, W], f32, name=f"pv{pair}_{sc}",
                                 tag="apv", bufs=2)
                    dn = ps.tile([128, W], f32, name=f"dn{pair}_{sc}",
                                 tag="adn", bufs=2)
                    for jb in range(4 * sc + 4):
                        ex = self.scores_exp(sc, jb)
                        if self.prev is not None:
                            self.pv_dn(self.prev)
                        self.prev = (pv, dn, sc, jb, ex)
                        filler_tick()

                def flush(self):
                    if self.prev is not None:
                        self.pv_dn(self.prev)
                        self.prev = None

            def attention(pair_obj, sc, filler_tick):
                pair_obj.window(sc, filler_tick)

            # ---- orchestration ----
            # Only the first window's q/k projection precedes attention;
            # later windows' projections ride inside earlier windows, so
            # the exp stream starts as soon as xt lands (~24us).
            proj_qk_unit(0, 0)
            for st in range(4):
                proj_v(st)

            def make_tick(queue, period):
                ticks = [0]

                def tick():
                    ticks[0] += 1
                    if ticks[0] % period == 0 and queue:
                        queue.pop(0)()
                return tick

            pa0 = PairAttention(0)
            for sc in range(NSC):
                q = []
                if sc + 1 < NSC:
                    q.append(lambda s=sc + 1: proj_qk_unit(0, s))
                    q += [(lambda st=st: proj_v(st))
                          for st in range(4 * sc + 4, 4 * sc + 8)]
                    period = 1 if sc == 0 else 2
                else:
                    q = [(lambda s=s: proj_qk_unit(1, s)) for s in range(NSC)]
                    period = 4
                attention(pa0, sc, make_tick(q, period))
                while q:
                    q.pop(0)()
            pa0.flush()
            pa1 = PairAttention(1)
            for sc in range(NSC):
                if sc < NSC - 1:
                    attention(pa1, sc, lambda: None)
                else:
                    q = [(lambda s=s: proj_qk_unit(2, s)) for s in range(NSC)]
                    attention(pa1, sc, make_tick(q, 4))
                    while q:
                        q.pop(0)()
            pa1.flush()
            pa2 = PairAttention(2)
            emitted = [0]
            for sc in range(NSC):
                allowed = max(0, 4 * sc)
                ticks = [0]

                def tick(allowed=allowed, ticks=ticks):
                    ticks[0] += 1
                    if ticks[0] % 3 == 0 and emitted[0] < min(allowed, 12):
                        outproj(emitted[0])
                        emitted[0] += 1

                attention(pa2, sc, tick)
            pa2.flush()
            for st in range(emitted[0], NST):
                outproj(st)

    _split_waits(nc)
    return nc


def _get_program():
    global _PROGRAM
    if _PROGRAM is None:
        _PROGRAM = _build_program()
    return _PROGRAM


def kernel(x, Wq, Wk, Wv, Wo, bo):
    global LAST_RESULT
    from concourse.bass_utils import run_bass_kernel_spmd

    x = np.asarray(x, np.float32)
    Wq = np.asarray(Wq, np.float32)
    Wk = np.asarray(Wk, np.float32)
    Wv = np.asarray(Wv, np.float32)
    Wo = np.asarray(Wo, np.float32)
    bo = np.asarray(bo, np.float32)

    tri = np.tril(np.ones((128, 128), np.float32)).T  # 1 where j<=i
    mk = tri.astype(np.float16)

    in_maps = []
    for c in range(8):
        b, g = divmod(c, 2)
        hs = slice(G * g, G * (g + 1))
        in_maps.append({
            "xt": np.ascontiguousarray(x[b].T).astype(np.float16),
            "wq": np.ascontiguousarray(Wq[hs, :].T).astype(np.float16),
            "wk": np.ascontiguousarray(Wk[hs, :].T).astype(np.float16),
            "wv": np.ascontiguousarray(Wv[hs, :].T).astype(np.float16),
            "wo": np.ascontiguousarray(Wo[:, hs].T).astype(np.float16),
            "mk": mk,
        })

    if PROFILE:
        _install_profile_hooks()
    nc = _get_program()
    res = run_bass_kernel_spmd(nc, in_maps, core_ids=list(range(8)),
                               trace=PROFILE, tmpdir=PROFILE_DIR)
    LAST_RESULT = res
    parts = [np.asarray(res.results[c]["y"], np.float32) for c in range(8)]
    out = np.stack([parts[2 * b] + parts[2 * b + 1] + bo for b in range(B)])
    return out.astype(np.float32)



# revision 10
# speedup vs baseline: 1.0309x; 1.0309x over previous
"""Causal multi-head attention block (B=4, S=2048, D=768, H=12, Dh=64)
distributed over 8 NeuronCores: core = (batch, head-group), each core
computes its 6 heads end-to-end plus its partial output projection;
host sums the two partials per batch and adds the bias.

Self-contained: hardcodes all shapes; no sibling imports.
"""

import numpy as np

B, S, D = 4, 2048, 768
H, DH = 12, 64
G = 384          # channels per head group (6 heads)
NPAIR = 3        # head pairs per core
NSC = 4          # 512-wide query windows
W = 512
NST = 16         # 128-row s-tiles
NDC = 6          # 128-row D chunks

_PROGRAM = None
PROFILE = False
PROFILE_DIR = None
LAST_RESULT = None


def _split_waits(nc, max_waits=1, max_updates=1):
    """This container's walrus rejects instructions carrying more than one
    semaphore wait/update ("Too many sync wait commands").  Move excess
    waits onto NoOps inserted before the owning instruction (same engine)
    and excess updates onto NoOps inserted after."""
    import concourse.mybir as mybir

    counter = [0]

    def nop(engine, waits, updates):
        counter[0] += 1
        n = mybir.InstNoOp(name=f"wsplit_nop_{counter[0]}", ins=[], outs=[])
        n.engine = engine
        n.sync_info = mybir.SyncInfo(on_wait=waits, on_update=updates)
        return n

    for bb in nc.main_func.blocks:
        out = []
        changed = False
        for ins in bb.instructions:
            si = ins.sync_info
            waits = list(si.on_wait) if si and si.on_wait else []
            updates = list(si.on_update) if si and si.on_update else []
            pre, post = [], []
            if len(waits) > max_waits:
                keep = waits[:max_waits - 1] if max_waits > 1 else []
                rest = waits[len(keep):]
                while rest:
                    chunk, rest = rest[:max_waits], rest[max_waits:]
                    pre.append(chunk)
                waits = keep
                changed = True
            if len(updates) > max_updates:
                rest = updates[max_updates:]
                updates = updates[:max_updates]
                while rest:
                    chunk, rest = rest[:max_updates], rest[max_updates:]
                    post.append(chunk)
                changed = True
            if pre or post:
                ins.sync_info = mybir.SyncInfo(
                    on_wait=waits, on_update=updates)
            for w in pre:
                out.append(nop(ins.engine, w, []))
            out.append(ins)
            for u in post:
                out.append(nop(ins.engine, [], u))
        if changed:
            bb.instructions = out


def _install_profile_hooks():
    """Dev-only (PROFILE=True): register the NTFF profile hook that the
    agent image's antenv lacks, and stub out the artifact upload."""
    import sys
    import types

    try:
        from antenv.axon_hooks import get_axon_ntff_profile_hook  # noqa: F401
    except ImportError:
        import antenv
        from trn_agent_boot import trn_boot

        hook = trn_boot._ntff_profile_via_ctypes("/opt/axon/libaxon_pjrt.so")
        mod = types.ModuleType("antenv.axon_hooks")
        mod._hook = hook
        mod.get_axon_ntff_profile_hook = lambda: mod._hook
        mod.set_axon_ntff_profile_hook = lambda h: setattr(mod, "_hook", h)
        sys.modules["antenv.axon_hooks"] = mod
        antenv.axon_hooks = mod

    from concourse import bass_utils

    bass_utils.upload_artifacts = lambda tmpdir: "local://" + tmpdir


def _build_program():
    import concourse.bass as bass
    import concourse.mybir as mybir
    import concourse.tile as tile

    f16 = mybir.dt.float16
    f32 = mybir.dt.float32

    nc = bass.Bass()
    xt_d = nc.declare_dram_parameter("xt", [D, S], f16, isOutput=False)
    wq_d = nc.declare_dram_parameter("wq", [D, G], f16, isOutput=False)
    wk_d = nc.declare_dram_parameter("wk", [D, G], f16, isOutput=False)
    wv_d = nc.declare_dram_parameter("wv", [D, G], f16, isOutput=False)
    wo_d = nc.declare_dram_parameter("wo", [G, D], f16, isOutput=False)
    mk_d = nc.declare_dram_parameter("mk", [128, 128], f16, isOutput=False)
    y_d = nc.declare_dram_parameter("y", [S, D], f16, isOutput=True)

    with tile.TileContext(nc) as tc:
        with (
            tc.tile_pool(name="const", bufs=1) as const,
            tc.tile_pool(name="work", bufs=3) as work,
            tc.tile_pool(name="outp", bufs=3) as outp,
            tc.tile_pool(name="ps", bufs=2, space="PSUM") as ps,
        ):
            # ---- persistent SBUF tiles ----
            xt = [const.tile([128, S], f16, name=f"xt{i}", tag=f"xt{i}")
                  for i in range(NDC)]
            wq = [const.tile([128, G], f16, name=f"wq{i}", tag=f"wq{i}")
                  for i in range(NDC)]
            wk = [const.tile([128, G], f16, name=f"wk{i}", tag=f"wk{i}")
                  for i in range(NDC)]
            wv = [const.tile([128, G], f16, name=f"wv{i}", tag=f"wv{i}")
                  for i in range(NDC)]
            wo = [const.tile([128, D], f16, name=f"wo{i}", tag=f"wo{i}")
                  for i in range(3)]
            qt = [const.tile([128, S], f16, name=f"qt{p}", tag=f"qt{p}")
                  for p in range(NPAIR)]
            kt = [const.tile([128, S], f16, name=f"kt{p}", tag=f"kt{p}")
                  for p in range(NPAIR)]
            vt = [const.tile([128, G], f16, name=f"vt{t}", tag=f"vt{t}")
                  for t in range(NST)]
            gt = [const.tile([128, S], f16, name=f"gt{p}", tag=f"gt{p}")
                  for p in range(NPAIR)]
            mk = const.tile([128, 128], f16, name="mk", tag="mk")
            ones = const.tile([128, DH], f16, name="ones", tag="ones")

            # ---- input DMAs, spread over three queues (sync + scalar HW
            # DGE, gpsimd SW DGE) in need-by order: window-0 inputs (wq,
            # xt cols 0:512, wk, mk) first so the first projection starts
            # ~6us in; later windows' columns stream behind them. ----
            def xq(i, a, b):  # xt column slice loader
                return dict(out=xt[i][:, a:b],
                            in_=xt_d[128 * i:128 * (i + 1), a:b])

            nc.sync.dma_start(out=mk, in_=mk_d[:, :])
            for i in range(NDC):
                nc.sync.dma_start(**xq(i, 0, 512))
                nc.scalar.dma_start(out=wq[i], in_=wq_d[128 * i:128 * (i + 1), :])
                nc.gpsimd.dma_start(out=wk[i], in_=wk_d[128 * i:128 * (i + 1), :])
            for i in range(NDC):
                nc.scalar.dma_start(out=wv[i], in_=wv_d[128 * i:128 * (i + 1), :])
                nc.gpsimd.dma_start(**xq(i, 512, 1024))
            for i in range(NDC):
                nc.sync.dma_start(**xq(i, 1024, 1536))
                nc.scalar.dma_start(**xq(i, 1536, 2048))
            for i in range(3):
                nc.gpsimd.dma_start(out=wo[i], in_=wo_d[128 * i:128 * (i + 1), :])
            nc.vector.memset(ones, 1.0)

            def act_copy(out, in_):
                # ScalarE Copy ('copy' is in every act table set, so no
                # table swaps): PSUM->SBUF projection copies ride the ACT
                # queue instead of DVE, where they'd queue behind the
                # window-end reciprocal and hold "sc" PSUM slots.
                nc.scalar.activation(
                    out=out, in_=in_,
                    func=mybir.ActivationFunctionType.Copy, scale=1.0)

            def proj_qk_unit(pair, sc):
                qp = ps.tile([128, W], f32, name=f"qp{pair}_{sc}",
                             tag="sc", bufs=2)
                for dc in range(NDC):
                    nc.tensor.matmul(
                        qp,
                        wq[dc][:, 128 * pair:128 * (pair + 1)],
                        xt[dc][:, W * sc:W * (sc + 1)],
                        start=(dc == 0), stop=(dc == NDC - 1))
                act_copy(qt[pair][:, W * sc:W * (sc + 1)], qp)
                kp = ps.tile([128, W], f32, name=f"kp{pair}_{sc}",
                             tag="sc", bufs=2)
                for dc in range(NDC):
                    nc.tensor.matmul(
                        kp,
                        wk[dc][:, 128 * pair:128 * (pair + 1)],
                        xt[dc][:, W * sc:W * (sc + 1)],
                        start=(dc == 0), stop=(dc == NDC - 1))
                act_copy(kt[pair][:, W * sc:W * (sc + 1)], kp)

            def proj_v(st):
                vp = ps.tile([128, G], f32, name=f"vp{st}", tag="sc", bufs=2)
                for dc in range(NDC):
                    nc.tensor.matmul(
                        vp,
                        xt[dc][:, 128 * st:128 * (st + 1)],
                        wv[dc],
                        start=(dc == 0), stop=(dc == NDC - 1))
                act_copy(vt[st], vp)

            def outproj(st):
                o0 = ps.tile([128, G], f32, name=f"o0_{st}", tag="apv", bufs=2)
                for cc in range(3):
                    nc.tensor.matmul(
                        o0,
                        gt[cc][:, 128 * st:128 * (st + 1)],
                        wo[cc][:, 0:G],
                        start=(cc == 0), stop=(cc == 2))
                o1 = ps.tile([128, G], f32, name=f"o1_{st}", tag="adn", bufs=2)
                for cc in range(3):
                    nc.tensor.matmul(
                        o1,
                        gt[cc][:, 128 * st:128 * (st + 1)],
                        wo[cc][:, G:D],
                        start=(cc == 0), stop=(cc == 2))
                ob = outp.tile([128, D], f16, name=f"ob{st}", tag="ob", bufs=4)
                nc.vector.tensor_copy(out=ob[:, 0:G], in_=o0)
                nc.vector.tensor_copy(out=ob[:, G:D], in_=o1)
                eng = nc.sync if st % 2 == 0 else nc.gpsimd
                eng.dma_start(
                    out=y_d[128 * st:128 * (st + 1), :], in_=ob)

            class PairAttention:
                """Per-pair attention with a software pipeline that crosses
                window boundaries: pv/dn of group g are issued after the
                scores of group g+1 (even when g+1 is in the next query
                window), so neither the PE nor ACT drains at boundaries."""

                def __init__(self, pair):
                    self.pair = pair
                    self.prev = None

                def scores_exp(self, sc, jb):
                    pair = self.pair
                    col0 = max(0, 128 * jb - W * sc)
                    diag = jb >= 4 * sc
                    sct = ps.tile([128, 1024], f32, name=f"sc{pair}_{sc}_{jb}",
                                  tag="sc", bufs=2)
                    nc.tensor.matmul(
                        sct[:, col0:W],
                        kt[pair][0:64, 128 * jb:128 * (jb + 1)],
                        qt[pair][0:64, W * sc + col0:W * (sc + 1)],
                        start=True, stop=True)
                    nc.tensor.matmul(
                        sct[:, W:2 * W - col0],
                        kt[pair][64:128, 128 * jb:128 * (jb + 1)],
                        qt[pair][64:128, W * sc + col0:W * (sc + 1)],
                        start=True, stop=True)
                    ex = work.tile([128, 1024], f16, name=f"ex{pair}_{sc}_{jb}",
                                   tag="exp", bufs=6)
                    nc.scalar.activation(
                        out=ex[:, col0:2 * W - col0],
                        in_=sct[:, col0:2 * W - col0],
                        func=mybir.ActivationFunctionType.Exp, scale=0.125)
                    if diag:  # zero the j>i triangle of the diagonal block
                        # on Pool (gpsimd): keeps the jb-critical mask off the
                        # DVE queue, where the window-end reciprocal (3.3us)
                        # would delay it and starve the PE
                        nc.gpsimd.tensor_mul(
                            ex[:, col0:col0 + 128], ex[:, col0:col0 + 128], mk)
                        nc.gpsimd.tensor_mul(
                            ex[:, W:W + 128], ex[:, W:W + 128], mk)
                    return ex

                def pv_dn(self, state):
                    pair = self.pair
                    pv, dn, sc, jb, ex = state
                    col0 = max(0, 128 * jb - W * sc)
                    first, last = (jb == 0), (jb == 4 * sc + 3)
                    nc.tensor.matmul(
                        pv[0:64, col0:W],
                        vt[jb][:, 128 * pair:128 * pair + 64],
                        ex[:, col0:W],
                        start=first, stop=last)
                    nc.tensor.matmul(
                        pv[64:128, col0:W],
                        vt[jb][:, 128 * pair + 64:128 * (pair + 1)],
                        ex[:, W:2 * W - col0],
                        start=first, stop=last)
                    nc.tensor.matmul(
                        dn[0:64, col0:W],
                        ones,
                        ex[:, col0:W],
                        start=first, stop=last)
                    nc.tensor.matmul(
                        dn[64:128, col0:W],
                        ones,
                        ex[:, W:2 * W - col0],
                        start=first, stop=last)
                    if last:  # window complete: normalize into gt
                        rc = work.tile([128, W], f32, name=f"rc{pair}_{sc}",
                                       tag="rc", bufs=3)
                        nc.vector.reciprocal(out=rc, in_=dn)
                        nc.vector.tensor_mul(
                            gt[pair][:, W * sc:W * (sc + 1)], pv, rc)

                def window(self, sc, filler_tick):
                    pair = self.pair
                    pv = ps.tile([128, W], f32, name=f"pv{pair}_{sc}",
                                 tag="apv", bufs=2)
                    dn = ps.tile([128, W], f32, name=f"dn{pair}_{sc}",
                                 tag="adn", bufs=2)
                    for jb in range(4 * sc + 4):
                        ex = self.scores_exp(sc, jb)
                        if self.prev is not None:
                            self.pv_dn(self.prev)
                        self.prev = (pv, dn, sc, jb, ex)
                        filler_tick()

                def flush(self):
                    if self.prev is not None:
                        self.pv_dn(self.prev)
                        self.prev = None

            def attention(pair_obj, sc, filler_tick):
                pair_obj.window(sc, filler_tick)

            # ---- orchestration ----
            # Only the first window's q/k projection precedes attention;
            # later windows' projections ride inside earlier windows, so
            # the exp stream starts as soon as xt lands (~24us).
            proj_qk_unit(0, 0)
            for st in range(4):
                proj_v(st)

            def make_tick(queue, period):
                ticks = [0]

                def tick():
                    ticks[0] += 1
                    if ticks[0] % period == 0 and queue:
                        queue.pop(0)()
                return tick

            pa0 = PairAttention(0)
            for sc in range(NSC):
                q = []
                if sc + 1 < NSC:
                    q.append(lambda s=sc + 1: proj_qk_unit(0, s))
                    q += [(lambda st=st: proj_v(st))
                          for st in range(4 * sc + 4, 4 * sc + 8)]
                    period = 1 if sc == 0 else 2
                else:
                    q = [(lambda s=s: proj_qk_unit(1, s)) for s in range(NSC)]
                    period = 4
                attention(pa0, sc, make_tick(q, period))
                while q:
                    q.pop(0)()
            pa0.flush()
            pa1 = PairAttention(1)
            for sc in range(NSC):
                if sc < NSC - 1:
                    attention(pa1, sc, lambda: None)
                else:
                    q = [(lambda s=s: proj_qk_unit(2, s)) for s in range(NSC)]
                    attention(pa1, sc, make_tick(q, 4))
                    while q:
                        q.pop(0)()
            pa1.flush()
            pa2 = PairAttention(2)
            emitted = [0]
            for sc in range(NSC):
                allowed = max(0, 4 * sc)
                ticks = [0]

                def tick(allowed=allowed, ticks=ticks):
                    ticks[0] += 1
                    if ticks[0] % 3 == 0 and emitted[0] < min(allowed, 12):
                        outproj(emitted[0])
                        emitted[0] += 1

                attention(pa2, sc, tick)
            pa2.flush()
            for st in range(emitted[0], NST):
                outproj(st)

    _split_waits(nc)
    return nc


def _get_program():
    global _PROGRAM
    if _PROGRAM is None:
        _PROGRAM = _build_program()
    return _PROGRAM


def kernel(x, Wq, Wk, Wv, Wo, bo):
    global LAST_RESULT
    from concourse.bass_utils import run_bass_kernel_spmd

    x = np.asarray(x, np.float32)
    Wq = np.asarray(Wq, np.float32)
    Wk = np.asarray(Wk, np.float32)
    Wv = np.asarray(Wv, np.float32)
    Wo = np.asarray(Wo, np.float32)
    bo = np.asarray(bo, np.float32)

    tri = np.tril(np.ones((128, 128), np.float32)).T  # 1 where j<=i
    mk = tri.astype(np.float16)

    in_maps = []
    for c in range(8):
        b, g = divmod(c, 2)
        hs = slice(G * g, G * (g + 1))
        in_maps.append({
            "xt": np.ascontiguousarray(x[b].T).astype(np.float16),
            "wq": np.ascontiguousarray(Wq[hs, :].T).astype(np.float16),
            "wk": np.ascontiguousarray(Wk[hs, :].T).astype(np.float16),
            "wv": np.ascontiguousarray(Wv[hs, :].T).astype(np.float16),
            "wo": np.ascontiguousarray(Wo[:, hs].T).astype(np.float16),
            "mk": mk,
        })

    if PROFILE:
        _install_profile_hooks()
    nc = _get_program()
    res = run_bass_kernel_spmd(nc, in_maps, core_ids=list(range(8)),
                               trace=PROFILE, tmpdir=PROFILE_DIR)
    LAST_RESULT = res
    parts = [np.asarray(res.results[c]["y"], np.float32) for c in range(8)]
    out = np.stack([parts[2 * b] + parts[2 * b + 1] + bo for b in range(B)])
    return out.astype(np.float32)



# revision 11
# speedup vs baseline: 1.0571x; 1.0254x over previous
"""Causal multi-head attention block (B=4, S=2048, D=768, H=12, Dh=64)
distributed over 8 NeuronCores: core = (batch, head-group), each core
computes its 6 heads end-to-end plus its partial output projection;
host sums the two partials per batch and adds the bias.

Self-contained: hardcodes all shapes; no sibling imports.
"""

import numpy as np

B, S, D = 4, 2048, 768
H, DH = 12, 64
G = 384          # channels per head group (6 heads)
NPAIR = 3        # head pairs per core
NSC = 4          # 512-wide query windows
W = 512
NST = 16         # 128-row s-tiles
NDC = 6          # 128-row D chunks

_PROGRAM = None
PROFILE = False
PROFILE_DIR = None
LAST_RESULT = None


def _split_waits(nc, max_waits=1, max_updates=1):
    """This container's walrus rejects instructions carrying more than one
    semaphore wait/update ("Too many sync wait commands").  Move excess
    waits onto NoOps inserted before the owning instruction (same engine)
    and excess updates onto NoOps inserted after."""
    import concourse.mybir as mybir

    counter = [0]

    def nop(engine, waits, updates):
        counter[0] += 1
        n = mybir.InstNoOp(name=f"wsplit_nop_{counter[0]}", ins=[], outs=[])
        n.engine = engine
        n.sync_info = mybir.SyncInfo(on_wait=waits, on_update=updates)
        return n

    for bb in nc.main_func.blocks:
        out = []
        changed = False
        for ins in bb.instructions:
            si = ins.sync_info
            waits = list(si.on_wait) if si and si.on_wait else []
            updates = list(si.on_update) if si and si.on_update else []
            pre, post = [], []
            if len(waits) > max_waits:
                keep = waits[:max_waits - 1] if max_waits > 1 else []
                rest = waits[len(keep):]
                while rest:
                    chunk, rest = rest[:max_waits], rest[max_waits:]
                    pre.append(chunk)
                waits = keep
                changed = True
            if len(updates) > max_updates:
                rest = updates[max_updates:]
                updates = updates[:max_updates]
                while rest:
                    chunk, rest = rest[:max_updates], rest[max_updates:]
                    post.append(chunk)
                changed = True
            if pre or post:
                ins.sync_info = mybir.SyncInfo(
                    on_wait=waits, on_update=updates)
            for w in pre:
                out.append(nop(ins.engine, w, []))
            out.append(ins)
            for u in post:
                out.append(nop(ins.engine, [], u))
        if changed:
            bb.instructions = out


def _install_profile_hooks():
    """Dev-only (PROFILE=True): register the NTFF profile hook that the
    agent image's antenv lacks, and stub out the artifact upload."""
    import sys
    import types

    try:
        from antenv.axon_hooks import get_axon_ntff_profile_hook  # noqa: F401
    except ImportError:
        import antenv
        from trn_agent_boot import trn_boot

        hook = trn_boot._ntff_profile_via_ctypes("/opt/axon/libaxon_pjrt.so")
        mod = types.ModuleType("antenv.axon_hooks")
        mod._hook = hook
        mod.get_axon_ntff_profile_hook = lambda: mod._hook
        mod.set_axon_ntff_profile_hook = lambda h: setattr(mod, "_hook", h)
        sys.modules["antenv.axon_hooks"] = mod
        antenv.axon_hooks = mod

    from concourse import bass_utils

    bass_utils.upload_artifacts = lambda tmpdir: "local://" + tmpdir


def _build_program():
    import concourse.bass as bass
    import concourse.mybir as mybir
    import concourse.tile as tile

    f16 = mybir.dt.float16
    f32 = mybir.dt.float32

    nc = bass.Bass()
    xt_d = nc.declare_dram_parameter("xt", [128, NDC, S], f16, isOutput=False)
    wq_d = nc.declare_dram_parameter("wq", [128, NDC, G], f16, isOutput=False)
    wk_d = nc.declare_dram_parameter("wk", [128, NDC, G], f16, isOutput=False)
    wv_d = nc.declare_dram_parameter("wv", [128, NDC, G], f16, isOutput=False)
    wo_d = nc.declare_dram_parameter("wo", [128, 3, D], f16, isOutput=False)
    mk_d = nc.declare_dram_parameter("mk", [128, 128], f16, isOutput=False)
    y_d = nc.declare_dram_parameter("y", [S, D], f16, isOutput=True)

    with tile.TileContext(nc) as tc:
        with (
            tc.tile_pool(name="const", bufs=1) as const,
            tc.tile_pool(name="work", bufs=3) as work,
            tc.tile_pool(name="outp", bufs=3) as outp,
            tc.tile_pool(name="ps", bufs=2, space="PSUM") as ps,
        ):
            # ---- persistent SBUF tiles ----
            # consolidated [128, chunk, cols] layouts: one DMA per tensor
            # (or per xt column-window) -- each dma_start trigger costs
            # ~600ns on its issuing engine and ~us-scale queue overhead,
            # so fewer+bigger transfers shorten the startup critically.
            xtb = const.tile([128, NDC, S], f16, name="xtb", tag="xtb")
            wqb = const.tile([128, NDC, G], f16, name="wqb", tag="wqb")
            wkb = const.tile([128, NDC, G], f16, name="wkb", tag="wkb")
            wvb = const.tile([128, NDC, G], f16, name="wvb", tag="wvb")
            wob = const.tile([128, 3, D], f16, name="wob", tag="wob")
            xt = [xtb[:, i, :] for i in range(NDC)]
            wq = [wqb[:, i, :] for i in range(NDC)]
            wk = [wkb[:, i, :] for i in range(NDC)]
            wv = [wvb[:, i, :] for i in range(NDC)]
            wo = [wob[:, i, :] for i in range(3)]
            qt = [const.tile([128, S], f16, name=f"qt{p}", tag=f"qt{p}")
                  for p in range(NPAIR)]
            kt = [const.tile([128, S], f16, name=f"kt{p}", tag=f"kt{p}")
                  for p in range(NPAIR)]
            vt = [const.tile([128, G], f16, name=f"vt{t}", tag=f"vt{t}")
                  for t in range(NST)]
            gt = [const.tile([128, S], f16, name=f"gt{p}", tag=f"gt{p}")
                  for p in range(NPAIR)]
            mk = const.tile([128, 128], f16, name="mk", tag="mk")
            ones = const.tile([128, DH], f16, name="ones", tag="ones")

            # ---- input DMAs: 10 triggers total, none on the scalar
            # engine (its queue must stay clear for copies+exps). sync
            # carries xt in window order; gpsimd (SW DGE) the weights. ----
            nc.sync.dma_start(out=mk, in_=mk_d[:, :])
            nc.gpsimd.dma_start(out=wqb, in_=wq_d[:, :, :])
            nc.gpsimd.dma_start(out=wkb, in_=wk_d[:, :, :])
            for w in range(NSC):
                nc.sync.dma_start(out=xtb[:, :, W * w:W * (w + 1)],
                                  in_=xt_d[:, :, W * w:W * (w + 1)])
            nc.gpsimd.dma_start(out=wvb, in_=wv_d[:, :, :])
            nc.gpsimd.dma_start(out=wob, in_=wo_d[:, :, :])
            nc.vector.memset(ones, 1.0)

            def act_copy(out, in_):
                # ScalarE Copy ('copy' is in every act table set, so no
                # table swaps): PSUM->SBUF projection copies ride the ACT
                # queue instead of DVE, where they'd queue behind the
                # window-end reciprocal and hold "sc" PSUM slots.
                nc.scalar.activation(
                    out=out, in_=in_,
                    func=mybir.ActivationFunctionType.Copy, scale=1.0)

            def proj_qk_unit(pair, sc):
                qp = ps.tile([128, W], f32, name=f"qp{pair}_{sc}",
                             tag="sc", bufs=2)
                for dc in range(NDC):
                    nc.tensor.matmul(
                        qp,
                        wq[dc][:, 128 * pair:128 * (pair + 1)],
                        xt[dc][:, W * sc:W * (sc + 1)],
                        start=(dc == 0), stop=(dc == NDC - 1))
                act_copy(qt[pair][:, W * sc:W * (sc + 1)], qp)
                kp = ps.tile([128, W], f32, name=f"kp{pair}_{sc}",
                             tag="sc", bufs=2)
                for dc in range(NDC):
                    nc.tensor.matmul(
                        kp,
                        wk[dc][:, 128 * pair:128 * (pair + 1)],
                        xt[dc][:, W * sc:W * (sc + 1)],
                        start=(dc == 0), stop=(dc == NDC - 1))
                act_copy(kt[pair][:, W * sc:W * (sc + 1)], kp)

            def proj_v(st):
                vp = ps.tile([128, G], f32, name=f"vp{st}", tag="sc", bufs=2)
                for dc in range(NDC):
                    nc.tensor.matmul(
                        vp,
                        xt[dc][:, 128 * st:128 * (st + 1)],
                        wv[dc],
                        start=(dc == 0), stop=(dc == NDC - 1))
                act_copy(vt[st], vp)

            def outproj(st):
                o0 = ps.tile([128, G], f32, name=f"o0_{st}", tag="apv", bufs=2)
                for cc in range(3):
                    nc.tensor.matmul(
                        o0,
                        gt[cc][:, 128 * st:128 * (st + 1)],
                        wo[cc][:, 0:G],
                        start=(cc == 0), stop=(cc == 2))
                o1 = ps.tile([128, G], f32, name=f"o1_{st}", tag="adn", bufs=2)
                for cc in range(3):
                    nc.tensor.matmul(
                        o1,
                        gt[cc][:, 128 * st:128 * (st + 1)],
                        wo[cc][:, G:D],
                        start=(cc == 0), stop=(cc == 2))
                ob = outp.tile([128, D], f16, name=f"ob{st}", tag="ob", bufs=4)
                nc.vector.tensor_copy(out=ob[:, 0:G], in_=o0)
                nc.vector.tensor_copy(out=ob[:, G:D], in_=o1)
                eng = nc.sync if st % 2 == 0 else nc.gpsimd
                eng.dma_start(
                    out=y_d[128 * st:128 * (st + 1), :], in_=ob)

            class PairAttention:
                """Per-pair attention with a software pipeline that crosses
                window boundaries: pv/dn of group g are issued after the
                scores of group g+1 (even when g+1 is in the next query
                window), so neither the PE nor ACT drains at boundaries."""

                def __init__(self, pair):
                    self.pair = pair
                    self.prev = None

                def scores_exp(self, sc, jb):
                    pair = self.pair
                    col0 = max(0, 128 * jb - W * sc)
                    diag = jb >= 4 * sc
                    sct = ps.tile([128, 1024], f32, name=f"sc{pair}_{sc}_{jb}",
                                  tag="sc", bufs=2)
                    nc.tensor.matmul(
                        sct[:, col0:W],
                        kt[pair][0:64, 128 * jb:128 * (jb + 1)],
                        qt[pair][0:64, W * sc + col0:W * (sc + 1)],
                        start=True, stop=True)
                    nc.tensor.matmul(
                        sct[:, W:2 * W - col0],
                        kt[pair][64:128, 128 * jb:128 * (jb + 1)],
                        qt[pair][64:128, W * sc + col0:W * (sc + 1)],
                        start=True, stop=True)
                    ex = work.tile([128, 1024], f16, name=f"ex{pair}_{sc}_{jb}",
                                   tag="exp", bufs=6)
                    nc.scalar.activation(
                        out=ex[:, col0:2 * W - col0],
                        in_=sct[:, col0:2 * W - col0],
                        func=mybir.ActivationFunctionType.Exp, scale=0.125)
                    if diag:  # zero the j>i triangle of the diagonal block
                        # on Pool (gpsimd): keeps the jb-critical mask off the
                        # DVE queue, where the window-end reciprocal (3.3us)
                        # would delay it and starve the PE
                        nc.gpsimd.tensor_mul(
                            ex[:, col0:col0 + 128], ex[:, col0:col0 + 128], mk)
                        nc.gpsimd.tensor_mul(
                            ex[:, W:W + 128], ex[:, W:W + 128], mk)
                    return ex

                def pv_dn(self, state):
                    pair = self.pair
                    pv, dn, sc, jb, ex = state
                    col0 = max(0, 128 * jb - W * sc)
                    first, last = (jb == 0), (jb == 4 * sc + 3)
                    nc.tensor.matmul(
                        pv[0:64, col0:W],
                        vt[jb][:, 128 * pair:128 * pair + 64],
                        ex[:, col0:W],
                        start=first, stop=last)
                    nc.tensor.matmul(
                        pv[64:128, col0:W],
                        vt[jb][:, 128 * pair + 64:128 * (pair + 1)],
                        ex[:, W:2 * W - col0],
                        start=first, stop=last)
                    nc.tensor.matmul(
                        dn[0:64, col0:W],
                        ones,
                        ex[:, col0:W],
                        start=first, stop=last)
                    nc.tensor.matmul(
                        dn[64:128, col0:W],
                        ones,
                        ex[:, W:2 * W - col0],
                        start=first, stop=last)
                    if last:  # window complete: normalize into gt
                        rc = work.tile([128, W], f32, name=f"rc{pair}_{sc}",
                                       tag="rc", bufs=3)
                        nc.vector.reciprocal(out=rc, in_=dn)
                        nc.vector.tensor_mul(
                            gt[pair][:, W * sc:W * (sc + 1)], pv, rc)

                def window(self, sc, filler_tick):
                    pair = self.pair
                    pv = ps.tile([128, W], f32, name=f"pv{pair}_{sc}",
                                 tag="apv", bufs=2)
                    dn = ps.tile([128, W], f32, name=f"dn{pair}_{sc}",
                                 tag="adn", bufs=2)
                    for jb in range(4 * sc + 4):
                        ex = self.scores_exp(sc, jb)
                        if self.prev is not None:
                            self.pv_dn(self.prev)
                        self.prev = (pv, dn, sc, jb, ex)
                        filler_tick()

                def flush(self):
                    if self.prev is not None:
                        self.pv_dn(self.prev)
                        self.prev = None

            def attention(pair_obj, sc, filler_tick):
                pair_obj.window(sc, filler_tick)

            # ---- orchestration ----
            # Only the first window's q/k projection precedes attention;
            # later windows' projections ride inside earlier windows, so
            # the exp stream starts as soon as xt lands (~24us).
            proj_qk_unit(0, 0)
            for st in range(4):
                proj_v(st)

            def make_tick(queue, period):
                ticks = [0]

                def tick():
                    ticks[0] += 1
                    if ticks[0] % period == 0 and queue:
                        queue.pop(0)()
                return tick

            pa0 = PairAttention(0)
            for sc in range(NSC):
                q = []
                if sc + 1 < NSC:
                    q.append(lambda s=sc + 1: proj_qk_unit(0, s))
                    q += [(lambda st=st: proj_v(st))
                          for st in range(4 * sc + 4, 4 * sc + 8)]
                    period = 1 if sc == 0 else 2
                else:
                    q = [(lambda s=s: proj_qk_unit(1, s)) for s in range(NSC)]
                    period = 4
                attention(pa0, sc, make_tick(q, period))
                while q:
                    q.pop(0)()
            pa0.flush()
            pa1 = PairAttention(1)
            for sc in range(NSC):
                if sc < NSC - 1:
                    attention(pa1, sc, lambda: None)
                else:
                    q = [(lambda s=s: proj_qk_unit(2, s)) for s in range(NSC)]
                    attention(pa1, sc, make_tick(q, 4))
                    while q:
                        q.pop(0)()
            pa1.flush()
            pa2 = PairAttention(2)
            emitted = [0]
            for sc in range(NSC):
                allowed = max(0, 4 * sc)
                ticks = [0]

                def tick(allowed=allowed, ticks=ticks):
                    ticks[0] += 1
                    if ticks[0] % 3 == 0 and emitted[0] < min(allowed, 12):
                        outproj(emitted[0])
                        emitted[0] += 1

                attention(pa2, sc, tick)
            pa2.flush()
            for st in range(emitted[0], NST):
                outproj(st)

    _split_waits(nc)
    return nc


def _get_program():
    global _PROGRAM
    if _PROGRAM is None:
        _PROGRAM = _build_program()
    return _PROGRAM


def kernel(x, Wq, Wk, Wv, Wo, bo):
    global LAST_RESULT
    from concourse.bass_utils import run_bass_kernel_spmd

    x = np.asarray(x, np.float32)
    Wq = np.asarray(Wq, np.float32)
    Wk = np.asarray(Wk, np.float32)
    Wv = np.asarray(Wv, np.float32)
    Wo = np.asarray(Wo, np.float32)
    bo = np.asarray(bo, np.float32)

    tri = np.tril(np.ones((128, 128), np.float32)).T  # 1 where j<=i
    mk = tri.astype(np.float16)

    in_maps = []
    for c in range(8):
        b, g = divmod(c, 2)
        hs = slice(G * g, G * (g + 1))
        def chunked(a, n):  # [n*128, M] -> [128, n, M]
            m = a.shape[1]
            return np.ascontiguousarray(
                a.reshape(n, 128, m).transpose(1, 0, 2)).astype(np.float16)

        in_maps.append({
            "xt": chunked(x[b].T, NDC),
            "wq": chunked(Wq[hs, :].T, NDC),
            "wk": chunked(Wk[hs, :].T, NDC),
            "wv": chunked(Wv[hs, :].T, NDC),
            "wo": chunked(Wo[:, hs].T, 3),
            "mk": mk,
        })

    if PROFILE:
        _install_profile_hooks()
    nc = _get_program()
    res = run_bass_kernel_spmd(nc, in_maps, core_ids=list(range(8)),
                               trace=PROFILE, tmpdir=PROFILE_DIR)
    LAST_RESULT = res
    parts = [np.asarray(res.results[c]["y"], np.float32) for c in range(8)]
    out = np.stack([parts[2 * b] + parts[2 * b + 1] + bo for b in range(B)])
    return out.astype(np.float32)

